# revision 4
# baseline (speedup 1.0000x reference)
"""Trainium2 Bass kernel for a dense multi-head self-attention block.

Computation (matches torch/diffusers Attention with upcast softmax):
    q/k/v = hs @ W.T + b ; per-head scaled QK^T ; softmax ; PV ; out proj.
Shapes: hs [2, 2048, 1024], 16 heads x 64 dim, fp32 in/out.

Sharding: batch*head parallel over 8 cores. Core c owns heads {2c, 2c+1}
(feature slice c*128:(c+1)*128 of E) for both batches. The host pre-packs
hidden_states and weights into partition-major fp16 layouts (one contiguous
run per SBUF partition per DMA), so the device never transposes activations
and DMA descriptor counts stay low. Per core:
  - Q^T/K^T/V^T projections for its 128 features over all 4096 tokens
    (fp16 operands, fp32 PSUM accumulation),
  - V^T is re-tiled to [tokens, features] via PE transposes; an all-ones
    column is appended so the PV matmul also accumulates the softmax
    denominator (row 64 of each PV accumulator),
  - attention in scores^T layout (K @ Q^T: k-tokens on partitions, q on
    the free dim); QK row-alternates the two heads so LDWEIGHTS hides
    under the previous matmul (PE is N-cycle-bound: ~0.50 ns per moving
    row at 512-wide PSUM tiles),
  - softmax exp is split across two engines to keep it off the PE's
    critical path: ~2/3 of (kt, head) tiles run exact exp on ScalarE
    (scale folded in); ~1/3 run on DVE as a Schraudolph-style bit-trick:
    i16 = round(s * 1024*log2(e)*SCALE + (15360 + c)), bitcast to fp16
    gives 2^x with <3% mantissa-interp error (end-to-end rel err ~6e-3,
    tolerance 2e-2). No max-subtraction: scores are O(1) by construction.
  - softmax normalization: the denominator row (PV row 64) is
    reciprocal'd in place on the same partition (free-dim offset), then
    DMA-broadcast to 64 partitions; fused into the PSUM->SBUF copy path,
  - partial out-projection (contraction over this core's 128 features)
    written as fp16 [4096, 1024]; the host sums the 8 partials + o_b.
"""

import numpy as np

import concourse.bass as bass
import concourse.mybir as mybir
import concourse.tile as tile
from concourse import bacc
from concourse.bass_utils import run_bass_kernel_spmd

B, S, E = 2, 2048, 1024
H, D = 16, 64
SCALE = D ** -0.5
NCORE = 8
T = B * S              # 4096 tokens
FPC = 128              # features per core (2 heads x 64)
HPC = 2                # heads per core

F32 = mybir.dt.float32
F16 = mybir.dt.float16
I16 = mybir.dt.int16
EXP = mybir.ActivationFunctionType.Exp
MULT = mybir.AluOpType.mult
ADD = mybir.AluOpType.add

# Schraudolph exp2 constants (fp16 bit trick), scale folded in:
#   i16 = s * SCALE * 1024/ln(2) + 15360 + c
SCH_A = SCALE * 1024.0 / float(np.log(2.0))
SCH_C = 15360.0 - 45.0

# set by test harness to profile; results stashed in LAST_RESULT
TRACE = False
DEBUG = False
LAST_RESULT = None
_CACHE = {}


def _exp_engine(kt, h):
    """Which engine computes exp for (kt, head) tile: ~1/3 DVE, rest ACT."""
    return "dve" if (kt * 2 + h) % 3 == 1 else "act"


def _build(ctx, tc, io):
    nc = tc.nc
    hs_p, wq_p, wk_p, wv_p, ow_t, out_p = (
        io["hs_p"], io["wq_p"], io["wk_p"], io["wv_p"], io["ow_t"], io["out_p"],
    )

    # ---------------- pools ----------------
    consts = ctx.enter_context(tc.tile_pool(name="consts", bufs=1))
    persist = ctx.enter_context(tc.tile_pool(name="persist", bufs=1))
    hst_pool = ctx.enter_context(tc.tile_pool(name="hst", bufs=4))
    vt_pool = ctx.enter_context(tc.tile_pool(name="vt", bufs=3))
    pt_pool = ctx.enter_context(tc.tile_pool(name="pt", bufs=6))
    bc_pool = ctx.enter_context(tc.tile_pool(name="bcs", bufs=3))
    rc_pool = ctx.enter_context(tc.tile_pool(name="rc", bufs=2))
    out_pool = ctx.enter_context(tc.tile_pool(name="outs", bufs=8))
    dr_pool = ctx.enter_context(tc.tile_pool(name="drb", bufs=2, space="DRAM"))
    # PSUM: 8 banks total. p_big = 2x[128,1024] (4 banks),
    # p_acc = 4x[128,512] (4 banks)
    p_big = ctx.enter_context(tc.tile_pool(name="p_big", bufs=2, space="PSUM"))
    p_acc = ctx.enter_context(tc.tile_pool(name="p_acc", bufs=4, space="PSUM"))

    # ---------------- constants / weights ----------------
    wq_sb = consts.tile([128, 8, 128], F16, tag="wq")
    wk_sb = consts.tile([128, 8, 128], F16, tag="wk")
    wv_sb = consts.tile([128, 8, 128], F16, tag="wv")
    ow_sb = consts.tile([128, 1024], F16, tag="ow")
    bias_sb = consts.tile([128, 3], F32, tag="bias")
    qb_sb, kb_sb, vb_sb = bias_sb[:, 0:1], bias_sb[:, 1:2], bias_sb[:, 2:3]
    cpack = consts.tile([128, 144], F16, tag="cpack")
    ident = cpack[:, 0:128]

    nc.sync.dma_start(wq_sb[:], wq_p[:])
    nc.sync.dma_start(wk_sb[:], wk_p[:])
    nc.sync.dma_start(wv_sb[:], wv_p[:])
    nc.sync.dma_start(bias_sb[:], io["bias3"][:])
    nc.sync.dma_start(cpack[:], io["cpack"][:])
    nc.sync.dma_start(ow_sb[:], ow_t[:])

    # persistent activations: feature dim (128 = 2 heads x 64) on partitions
    qt_sb = persist.tile([128, T], F16, tag="qt")      # Q^T
    kt_sb = persist.tile([128, T], F16, tag="kt")      # K^T
    at_sb = persist.tile([128, T], F16, tag="at")      # attn out^T (normalized)
    v_bh = [
        [
            persist.tile([128, 16, 65], F16, tag=f"v{b}{h}", name=f"v{b}{h}")
            for h in range(2)
        ]
        for b in range(B)
    ]
    # v_bh[b][h][:, kt, 0:64]: token kt*128+p of batch b, head-h features;
    # column 64 is all-ones (rides along in PV to accumulate softmax denom)
    for b in range(B):
        for h in range(2):
            nc.vector.tensor_copy(
                v_bh[b][h][:, :, 64:65],
                cpack[:, 128:144].rearrange("p (a o) -> p a o", o=1),
            )

    # ---------------- phase 1: QKV projections ----------------
    for tb in range(8):                      # 512-token blocks over B*S
        hst = hst_pool.tile([128, 8, 512], F16, tag="hst")
        nc.sync.dma_start(hst[:], hs_p[:, tb])
        for w_sb, b_sb, dest in ((wq_sb, qb_sb, qt_sb), (wk_sb, kb_sb, kt_sb)):
            ps = p_big.tile([128, 512], F32, tag="sc", name="ps")
            for et in range(8):
                nc.tensor.matmul(
                    ps[:], w_sb[:, et, :], hst[:, et, :],
                    start=(et == 0), stop=(et == 7),
                )
            nc.vector.tensor_scalar_add(
                dest[:, tb * 512:(tb + 1) * 512], ps[:], b_sb[:]
            )
        # V^T then transpose into [tokens, features] tiles
        vps = p_acc.tile([128, 512], F32, tag="acc")
        for et in range(8):
            nc.tensor.matmul(
                vps[:], wv_sb[:, et, :], hst[:, et, :],
                start=(et == 0), stop=(et == 7),
            )
        vtt = vt_pool.tile([128, 512], F16, tag="vtt")
        nc.vector.tensor_scalar_add(vtt[:], vps[:], vb_sb[:])
        b = tb // 4
        for j in range(4):
            ktl = (tb % 4) * 4 + j           # k-tile index within batch
            tps = p_acc.tile([128, 128], F16, tag="acc")
            nc.tensor.transpose(tps[:], vtt[:, j * 128:(j + 1) * 128], ident[:])
            nc.vector.tensor_copy(v_bh[b][0][:, ktl, 0:64], tps[:, 0:64])
            nc.vector.tensor_copy(v_bh[b][1][:, ktl, 0:64], tps[:, 64:128])

    # ---------------- phase 2: attention ----------------
    for b in range(B):
        toff = b * S
        for qb_i in range(2):                # 1024-wide q blocks
            qoff = toff + qb_i * 1024
            pv = [
                [
                    p_acc.tile([65, 512], F32, tag="acc", name=f"pv{h}{qs}")
                    for qs in range(2)
                ]
                for h in range(2)
            ]
            def emit_qk(kt):
                koff2 = toff + kt * 128
                sc = [
                    p_big.tile([128, 1024], F32, tag="sc", name=f"sc{h}")
                    for h in range(2)
                ]
                # alternate head row-groups so each LDWEIGHTS overlaps the
                # previous matmul (different row group -> PE pulls it ahead)
                for qs in range(2):
                    for h in range(2):
                        p0 = h * 64
                        nc.tensor.matmul(
                            sc[h][:, qs * 512:(qs + 1) * 512],
                            kt_sb[p0:p0 + 64, koff2:koff2 + 128],
                            qt_sb[p0:p0 + 64, qoff + qs * 512:qoff + (qs + 1) * 512],
                            start=True, stop=True,
                        )
                return sc

            sc_next = emit_qk(0)
            for kt in range(16):
                sc = sc_next
                pt = []
                for h in range(2):
                    pth = pt_pool.tile([128, 1024], F16, tag="pt")
                    if _exp_engine(kt, h) == "act":
                        nc.scalar.activation(pth[:], sc[h][:], EXP, scale=SCALE)
                    else:
                        nc.vector.tensor_scalar(
                            pth[:].bitcast(I16), sc[h][:], SCH_A, SCH_C,
                            MULT, ADD,
                        )
                    pt.append(pth)
                if kt < 15:
                    sc_next = emit_qk(kt + 1)
                first, last = kt == 0, kt == 15
                for qs in range(2):
                    q0, q1 = qs * 512, (qs + 1) * 512
                    # PV with ones-row: out rows 0:64 = V^T P^T, row 64 = denom
                    for h in range(2):
                        nc.tensor.matmul(
                            pv[h][qs][:], v_bh[b][h][:, kt, :],
                            pt[h][:, q0:q1], start=first, stop=last,
                        )
            # copy PV out of PSUM immediately (frees the accumulator banks
            # so the next q-block's matmuls can start), then normalize from
            # SBUF: at = pv[0:64] * (1 / pv[64]) broadcast over rows.
            # Layout: cols 0:2048 = pv data (h0 | h1), cols 2048:4096 =
            # reciprocal scratch on the same partition (row 64) so the DVE
            # reciprocal stays lane-aligned with the denominator row.
            pvs_all = rc_pool.tile([65, 4096], F32, tag="pvs", name="pvs_all")
            pvs = [pvs_all[:, 0:1024], pvs_all[:, 1024:2048]]
            for h in range(2):
                for qs in range(2):
                    nc.vector.tensor_copy(
                        pvs[h][:, qs * 512:(qs + 1) * 512], pv[h][qs][:]
                    )
            with nc.allow_low_precision(reason="softmax denom reciprocal"):
                nc.vector.reciprocal(
                    pvs_all[64:65, 2048:4096], pvs_all[64:65, 0:2048]
                )
            # DMA broadcast needs a DRAM source (SBUF APs can't have a
            # 0-stride partition dim): bounce the reciprocal row out once,
            # then broadcast-read it to 64 partitions per head.
            rcp_dr = dr_pool.tile([2, 1024], F32, tag="rcp_dr", name="rcp_dr")
            nc.sync.dma_start(
                rcp_dr.rearrange("a n -> (a n)"), pvs_all[64:65, 2048:4096]
            )
            bc = [None, None]
            for h in range(2):
                bch = bc_pool.tile([64, 1024], F32, tag="bcs", name=f"bc{h}")
                nc.sync.dma_start(bch[:], rcp_dr[h:h + 1, :].broadcast_to([64, 1024]))
                bc[h] = bch
            nc.vector.tensor_mul(
                at_sb[0:64, qoff:qoff + 1024], pvs[0][0:64, :], bc[0][:]
            )
            a1 = vt_pool.tile([64, 1024], F16, tag="a1", name="a1")
            nc.vector.tensor_mul(a1[:], pvs[1][0:64, :], bc[1][:])
            # head 1 lives on partitions 64:128 of at_sb -- shift via SBUF->SBUF DMA
            nc.sync.dma_start(at_sb[64:128, qoff:qoff + 1024], a1[:])

    if DEBUG:
        nc.sync.dma_start(io["dbg_qt"][:], qt_sb[:])
        nc.sync.dma_start(io["dbg_kt"][:], kt_sb[:])
        nc.sync.dma_start(io["dbg_at"][:], at_sb[:])
        nc.sync.dma_start(io["dbg_v00"][:], v_bh[0][0][:].rearrange("p a b -> p (a b)"))

    # ---------------- phase 3: partial out-projection ----------------
    # ops tiles alternate between both PSUM pools so up to 6 matmul/copy
    # pairs are in flight; copies alternate DVE / ScalarE.
    for tb in range(32):                     # 128-token blocks
        t0 = tb * 128
        ot = out_pool.tile([128, 1024], F16, tag="outs", name="ot")
        for eb in range(2):
            pool = p_acc if (tb * 2 + eb) % 3 != 2 else p_big
            ops = pool.tile([128, 512], F32, tag="acc" if pool is p_acc else "sc",
                            name="ops")
            nc.tensor.matmul(
                ops[:], at_sb[:, t0:t0 + 128],
                ow_sb[:, eb * 512:(eb + 1) * 512],
                start=True, stop=True,
            )
            if eb == 0:
                nc.vector.tensor_copy(ot[:, 0:512], ops[:])
            else:
                nc.scalar.copy(ot[:, 512:1024], ops[:])
        nc.sync.dma_start(out_p[t0:t0 + 128, :], ot[:])


def _get_program():
    if "nc" in _CACHE:
        return _CACHE["nc"]
    from contextlib import ExitStack

    nc = bacc.Bacc("TRN2", target_bir_lowering=False, debug=False,
                   num_devices=NCORE)
    io = {
        "hs_p": nc.dram_tensor("hs_p", [128, 8, 8, 512], F16, kind="ExternalInput").ap(),
        "wq_p": nc.dram_tensor("wq_p", [128, 8, 128], F16, kind="ExternalInput").ap(),
        "wk_p": nc.dram_tensor("wk_p", [128, 8, 128], F16, kind="ExternalInput").ap(),
        "wv_p": nc.dram_tensor("wv_p", [128, 8, 128], F16, kind="ExternalInput").ap(),
        "ow_t": nc.dram_tensor("ow_t", [FPC, E], F16, kind="ExternalInput").ap(),
        "bias3": nc.dram_tensor("bias3", [FPC, 3], F32, kind="ExternalInput").ap(),
        "cpack": nc.dram_tensor("cpack", [128, 144], F16, kind="ExternalInput").ap(),
        "out_p": nc.dram_tensor("out_p", [T, E], F16, kind="ExternalOutput").ap(),
    }
    if DEBUG:
        io["dbg_qt"] = nc.dram_tensor("dbg_qt", [128, T], F16, kind="ExternalOutput").ap()
        io["dbg_kt"] = nc.dram_tensor("dbg_kt", [128, T], F16, kind="ExternalOutput").ap()
        io["dbg_at"] = nc.dram_tensor("dbg_at", [128, T], F16, kind="ExternalOutput").ap()
        io["dbg_v00"] = nc.dram_tensor("dbg_v00", [128, 16 * 65], F16, kind="ExternalOutput").ap()
    with tile.TileContext(nc) as tc:
        with ExitStack() as ctx:
            _build(ctx, tc, io)
    nc.compile()
    _CACHE["nc"] = nc
    return nc


def kernel(hidden_states, q_w, q_b, k_w, k_b, v_w, v_b, o_w, o_b):
    global LAST_RESULT
    nc = _get_program()

    f32c = lambda a: np.ascontiguousarray(a, dtype=np.float32)
    f16c = lambda a: np.ascontiguousarray(a, dtype=np.float16)
    # hs_p[p, tb, et, n] = hs[token tb*512+n, feature et*128+p]
    hs_t = np.asarray(hidden_states, dtype=np.float32).reshape(T, E).T
    hs_pm = f16c(hs_t.reshape(8, 128, 8, 512).transpose(1, 2, 0, 3))
    wp = lambda w, sl: f16c(
        np.asarray(w)[sl, :].T.reshape(8, 128, FPC).transpose(1, 0, 2)
    )
    in_maps = []
    for c in range(NCORE):
        sl = slice(c * FPC, (c + 1) * FPC)
        in_maps.append({
            "hs_p": hs_pm,
            "wq_p": wp(q_w, sl),
            "wk_p": wp(k_w, sl),
            "wv_p": wp(v_w, sl),
            "ow_t": f16c(np.asarray(o_w)[:, sl].T),
            "bias3": f32c(np.stack([np.asarray(q_b)[sl], np.asarray(k_b)[sl],
                                     np.asarray(v_b)[sl]], axis=1)),
            "cpack": f16c(np.concatenate([np.eye(128, dtype=np.float16),
                                          np.ones((128, 16), np.float16)], axis=1)),
        })

    res = run_bass_kernel_spmd(nc, in_maps, list(range(NCORE)), trace=TRACE)
    LAST_RESULT = res
    out = res.results[0]["out_p"].astype(np.float64)
    for c in range(1, NCORE):
        out += res.results[c]["out_p"]
    out += np.asarray(o_b, dtype=np.float64)
    return out.reshape(B, S, E).astype(np.float32)


# revision 8
# speedup vs baseline: 1.2992x; 1.2992x over previous
"""Trainium2 Bass kernel for a dense multi-head self-attention block.

Computation (matches torch/diffusers Attention with upcast softmax):
    q/k/v = hs @ W.T + b ; per-head scaled QK^T ; softmax ; PV ; out proj.
Shapes: hs [2, 2048, 1024], 16 heads x 64 dim, fp32 in/out.

Sharding: batch*head parallel over 8 cores. Core c owns heads {2c, 2c+1}
(feature slice c*128:(c+1)*128 of E) for both batches. The host pre-packs
hidden_states and weights into partition-major fp16 layouts (one contiguous
run per SBUF partition per DMA), so the device never transposes activations
and DMA descriptor counts stay low. Per core:
  - Q^T/K^T/V^T projections for its 128 features over all 4096 tokens
    (fp16 operands, fp32 PSUM accumulation),
  - V^T is re-tiled to [tokens, features] via PE transposes; an all-ones
    column is appended so the PV matmul also accumulates the softmax
    denominator (row 64 of each PV accumulator),
  - attention in scores^T layout (K @ Q^T: k-tokens on partitions, q on
    the free dim); QK row-alternates the two heads so LDWEIGHTS hides
    under the previous matmul (PE is N-cycle-bound: ~0.50 ns per moving
    row at 512-wide PSUM tiles),
  - softmax exp is split across two engines to keep it off the PE's
    critical path: ~2/3 of (kt, head) tiles run exact exp on ScalarE
    (scale folded in); ~1/3 run on DVE as a Schraudolph-style bit-trick:
    i16 = round(s * 1024*log2(e)*SCALE + (15360 + c)), bitcast to fp16
    gives 2^x with <3% mantissa-interp error (end-to-end rel err ~6e-3,
    tolerance 2e-2). No max-subtraction: scores are O(1) by construction.
  - softmax normalization: the denominator row (PV row 64) is
    reciprocal'd in place on the same partition (free-dim offset), then
    DMA-broadcast to 64 partitions; fused into the PSUM->SBUF copy path,
  - partial out-projection (contraction over this core's 128 features)
    written as fp16 [4096, 1024]; the host sums the 8 partials + o_b.
"""

import numpy as np

import concourse.bass as bass
import concourse.mybir as mybir
import concourse.tile as tile
from concourse import bacc
from concourse.bass_utils import run_bass_kernel_spmd

B, S, E = 2, 2048, 1024
H, D = 16, 64
SCALE = D ** -0.5
NCORE = 8
T = B * S              # 4096 tokens
FPC = 128              # features per core (2 heads x 64)
HPC = 2                # heads per core

F32 = mybir.dt.float32
F16 = mybir.dt.float16
I16 = mybir.dt.int16
EXP = mybir.ActivationFunctionType.Exp
MULT = mybir.AluOpType.mult
ADD = mybir.AluOpType.add

# Schraudolph exp2 constants (fp16 bit trick), scale folded in:
#   i16 = s * SCALE * 1024/ln(2) + 15360 + c
SCH_A = SCALE * 1024.0 / float(np.log(2.0))
SCH_C = 15360.0 - 45.0

# set by test harness to profile; results stashed in LAST_RESULT
TRACE = False
DEBUG = False
LAST_RESULT = None
_CACHE = {}


def _exp_engine(kt, h):
    """Which engine computes exp for (kt, head) tile: ~1/3 DVE, rest ACT."""
    return "dve" if (kt * 2 + h) % 3 == 1 else "act"


def _build(ctx, tc, io):
    nc = tc.nc
    hs_p, wq_p, wk_p, wv_p, ow_t, out_p = (
        io["hs_p"], io["wq_p"], io["wk_p"], io["wv_p"], io["ow_t"], io["out_p"],
    )

    # ---------------- pools ----------------
    consts = ctx.enter_context(tc.tile_pool(name="consts", bufs=1))
    persist = ctx.enter_context(tc.tile_pool(name="persist", bufs=1))
    hst_pool = ctx.enter_context(tc.tile_pool(name="hst", bufs=4))
    vt_pool = ctx.enter_context(tc.tile_pool(name="vt", bufs=3))
    pt_pool = ctx.enter_context(tc.tile_pool(name="pt", bufs=6))
    bc_pool = ctx.enter_context(tc.tile_pool(name="bcs", bufs=3))
    rc_pool = ctx.enter_context(tc.tile_pool(name="rc", bufs=2))
    out_pool = ctx.enter_context(tc.tile_pool(name="outs", bufs=8))
    dr_pool = ctx.enter_context(tc.tile_pool(name="drb", bufs=2, space="DRAM"))
    # PSUM: 8 banks total. p_big = 2x[128,1024] (4 banks),
    # p_acc = 4x[128,512] (4 banks)
    p_big = ctx.enter_context(tc.tile_pool(name="p_big", bufs=2, space="PSUM"))
    p_acc = ctx.enter_context(tc.tile_pool(name="p_acc", bufs=4, space="PSUM"))

    # ---------------- constants / weights ----------------
    wq_sb = consts.tile([128, 8, 128], F16, tag="wq")
    wk_sb = consts.tile([128, 8, 128], F16, tag="wk")
    wv_sb = consts.tile([128, 8, 128], F16, tag="wv")
    ow_sb = consts.tile([128, 1024], F16, tag="ow")
    bias_sb = consts.tile([128, 3], F32, tag="bias")
    qb_sb, kb_sb, vb_sb = bias_sb[:, 0:1], bias_sb[:, 1:2], bias_sb[:, 2:3]
    cpack = consts.tile([128, 144], F16, tag="cpack")
    ident = cpack[:, 0:128]

    nc.sync.dma_start(wq_sb[:], wq_p[:])
    nc.sync.dma_start(wk_sb[:], wk_p[:])
    nc.sync.dma_start(wv_sb[:], wv_p[:])
    nc.sync.dma_start(bias_sb[:], io["bias3"][:])
    nc.sync.dma_start(cpack[:], io["cpack"][:])
    nc.sync.dma_start(ow_sb[:], ow_t[:])

    # persistent activations: feature dim (128 = 2 heads x 64) on partitions
    qt_sb = persist.tile([128, T], F16, tag="qt")      # Q^T
    kt_sb = persist.tile([128, T], F16, tag="kt")      # K^T
    # attn out^T (normalized), one tile per 1024-q block so the out-proj
    # only waits on the q-blocks it actually reads
    at4 = [
        persist.tile([128, 1024], F16, tag=f"at{i}", name=f"at{i}")
        for i in range(4)
    ]
    v_bh = [
        [
            persist.tile([128, 16, 65], F16, tag=f"v{b}{h}", name=f"v{b}{h}")
            for h in range(2)
        ]
        for b in range(B)
    ]
    # v_bh[b][h][:, kt, 0:64]: token kt*128+p of batch b, head-h features;
    # column 64 is all-ones (rides along in PV to accumulate softmax denom)
    for b in range(B):
        for h in range(2):
            nc.vector.tensor_copy(
                v_bh[b][h][:, :, 64:65],
                cpack[:, 128:144].rearrange("p (a o) -> p a o", o=1),
            )

    # ---------------- phase 1: QKV projections ----------------
    for tb in range(8):                      # 512-token blocks over B*S
        hst = hst_pool.tile([128, 8, 512], F16, tag="hst")
        nc.sync.dma_start(hst[:], hs_p[:, tb])
        for w_sb, b_sb, dest in ((wq_sb, qb_sb, qt_sb), (wk_sb, kb_sb, kt_sb)):
            ps = p_big.tile([128, 512], F32, tag="sc", name="ps")
            for et in range(8):
                nc.tensor.matmul(
                    ps[:], w_sb[:, et, :], hst[:, et, :],
                    start=(et == 0), stop=(et == 7),
                )
            nc.vector.tensor_scalar_add(
                dest[:, tb * 512:(tb + 1) * 512], ps[:], b_sb[:]
            )
        # V^T then transpose into [tokens, features] tiles
        vps = p_acc.tile([128, 512], F32, tag="acc")
        for et in range(8):
            nc.tensor.matmul(
                vps[:], wv_sb[:, et, :], hst[:, et, :],
                start=(et == 0), stop=(et == 7),
            )
        vtt = vt_pool.tile([128, 512], F16, tag="vtt")
        nc.vector.tensor_scalar_add(vtt[:], vps[:], vb_sb[:])
        b = tb // 4
        for j in range(4):
            ktl = (tb % 4) * 4 + j           # k-tile index within batch
            tps = p_acc.tile([128, 128], F16, tag="acc")
            nc.tensor.transpose(tps[:], vtt[:, j * 128:(j + 1) * 128], ident[:])
            nc.vector.tensor_copy(v_bh[b][0][:, ktl, 0:64], tps[:, 0:64])
            nc.vector.tensor_copy(v_bh[b][1][:, ktl, 0:64], tps[:, 64:128])

    # ---------------- phase 2: attention ----------------
    # The normalization chain of q-block N (reciprocal pack-dance, DMA
    # broadcast, muls) is deferred into q-block N+1's kt loop so its DMA
    # latency never blocks the DVE queue entries that feed the PE.
    def emit_norm(pvs_all, at_dst):
        pvs = [pvs_all[:, 0:1024], pvs_all[:, 1024:2048]]
        # Reciprocal of the 2048 denominators (2 heads x 1024 q).
        # DVE reciprocal costs ~6.3 ns per free-dim element regardless of
        # partition count, so pack them across 128 partitions via a DRAM
        # bounce: [1,2048] row -> [128,16] -> recip -> row -> broadcast.
        den_dr = dr_pool.tile([2, 1024], F32, tag="den_dr", name="den_dr")
        nc.sync.dma_start(den_dr.rearrange("a n -> (a n)"), pvs_all[64:65, :])
        dpack = rc_pool.tile([128, 16], F32, tag="rc", name="dpack")
        nc.sync.dma_start(
            dpack[:],
            den_dr.rearrange("a n -> (a n)").rearrange("(p i) -> p i", p=128),
        )
        rpack = rc_pool.tile([128, 16], F32, tag="rc", name="rpack")
        with nc.allow_low_precision(reason="softmax denom reciprocal"):
            nc.vector.reciprocal(rpack[:], dpack[:])
        rcp_dr = dr_pool.tile([2, 1024], F32, tag="rcp_dr", name="rcp_dr")
        nc.sync.dma_start(
            rcp_dr.rearrange("a n -> (a n)").rearrange("(p i) -> p i", p=128),
            rpack[:],
        )
        bc = [None, None]
        for h in range(2):
            bch = bc_pool.tile([64, 1024], F32, tag="bcs", name=f"bc{h}")
            nc.sync.dma_start(bch[:], rcp_dr[h:h + 1, :].broadcast_to([64, 1024]))
            bc[h] = bch
        nc.vector.tensor_mul(at_dst[0:64, :], pvs[0][0:64, :], bc[0][:])
        a1 = vt_pool.tile([64, 1024], F16, tag="a1", name="a1")
        nc.vector.tensor_mul(a1[:], pvs[1][0:64, :], bc[1][:])
        # head 1 lives on partitions 64:128 of at -- shift via SBUF->SBUF DMA
        nc.sync.dma_start(at_dst[64:128, :], a1[:])

    pending_norm = None
    for b in range(B):
        toff = b * S
        for qb_i in range(2):                # 1024-wide q blocks
            qbg = b * 2 + qb_i               # global q-block index
            qoff = toff + qb_i * 1024
            pv = [
                [
                    p_acc.tile([65, 512], F32, tag="acc", name=f"pv{h}{qs}")
                    for qs in range(2)
                ]
                for h in range(2)
            ]
            def emit_qk(kt):
                koff2 = toff + kt * 128
                sc = [
                    p_big.tile([128, 1024], F32, tag="sc", name=f"sc{h}")
                    for h in range(2)
                ]
                # alternate head row-groups so each LDWEIGHTS overlaps the
                # previous matmul (different row group -> PE pulls it ahead)
                for qs in range(2):
                    for h in range(2):
                        p0 = h * 64
                        nc.tensor.matmul(
                            sc[h][:, qs * 512:(qs + 1) * 512],
                            kt_sb[p0:p0 + 64, koff2:koff2 + 128],
                            qt_sb[p0:p0 + 64, qoff + qs * 512:qoff + (qs + 1) * 512],
                            start=True, stop=True,
                        )
                return sc

            sc_next = emit_qk(0)
            for kt in range(16):
                sc = sc_next
                pt = []
                for h in range(2):
                    pth = pt_pool.tile([128, 1024], F16, tag="pt")
                    if _exp_engine(kt, h) == "act":
                        nc.scalar.activation(pth[:], sc[h][:], EXP, scale=SCALE)
                    else:
                        nc.vector.tensor_scalar(
                            pth[:].bitcast(I16), sc[h][:], SCH_A, SCH_C,
                            MULT, ADD,
                        )
                    pt.append(pth)
                if kt == 3 and pending_norm is not None:
                    emit_norm(*pending_norm)
                    pending_norm = None
                if kt < 15:
                    sc_next = emit_qk(kt + 1)
                first, last = kt == 0, kt == 15
                for qs in range(2):
                    q0, q1 = qs * 512, (qs + 1) * 512
                    # PV with ones-row: out rows 0:64 = V^T P^T, row 64 = denom
                    for h in range(2):
                        nc.tensor.matmul(
                            pv[h][qs][:], v_bh[b][h][:, kt, :],
                            pt[h][:, q0:q1], start=first, stop=last,
                        )
            # copy PV out of PSUM immediately (frees the accumulator banks
            # for the next q-block); the normalization itself is deferred.
            pvs_all = rc_pool.tile([65, 2048], F32, tag="pvs", name="pvs_all")
            pvs = [pvs_all[:, 0:1024], pvs_all[:, 1024:2048]]
            for h in range(2):
                for qs in range(2):
                    nc.vector.tensor_copy(
                        pvs[h][:, qs * 512:(qs + 1) * 512], pv[h][qs][:]
                    )
            pending_norm = (pvs_all, at4[qbg])
    emit_norm(*pending_norm)
    pending_norm = None

    if DEBUG:
        nc.sync.dma_start(io["dbg_qt"][:], qt_sb[:])
        nc.sync.dma_start(io["dbg_kt"][:], kt_sb[:])
        for i in range(4):
            nc.sync.dma_start(io["dbg_at"][:, i * 1024:(i + 1) * 1024], at4[i][:])
        nc.sync.dma_start(io["dbg_v00"][:], v_bh[0][0][:].rearrange("p a b -> p (a b)"))

    # ---------------- phase 3: partial out-projection ----------------
    # ops tiles alternate between both PSUM pools so up to 6 matmul/copy
    # pairs are in flight; copies alternate DVE / ScalarE.
    for tb in range(32):                     # 128-token blocks
        t0 = tb * 128
        at_t = at4[tb // 8]
        c0 = (tb % 8) * 128
        ot = out_pool.tile([128, 1024], F16, tag="outs", name="ot")
        for eb in range(2):
            pool = p_acc if (tb * 2 + eb) % 3 != 2 else p_big
            ops = pool.tile([128, 512], F32, tag="acc" if pool is p_acc else "sc",
                            name="ops")
            nc.tensor.matmul(
                ops[:], at_t[:, c0:c0 + 128],
                ow_sb[:, eb * 512:(eb + 1) * 512],
                start=True, stop=True,
            )
            if eb == 0:
                nc.vector.tensor_copy(ot[:, 0:512], ops[:])
            else:
                nc.scalar.copy(ot[:, 512:1024], ops[:])
        nc.sync.dma_start(out_p[t0:t0 + 128, :], ot[:])


def _get_program():
    if "nc" in _CACHE:
        return _CACHE["nc"]
    from contextlib import ExitStack

    nc = bacc.Bacc("TRN2", target_bir_lowering=False, debug=False,
                   num_devices=NCORE)
    io = {
        "hs_p": nc.dram_tensor("hs_p", [128, 8, 8, 512], F16, kind="ExternalInput").ap(),
        "wq_p": nc.dram_tensor("wq_p", [128, 8, 128], F16, kind="ExternalInput").ap(),
        "wk_p": nc.dram_tensor("wk_p", [128, 8, 128], F16, kind="ExternalInput").ap(),
        "wv_p": nc.dram_tensor("wv_p", [128, 8, 128], F16, kind="ExternalInput").ap(),
        "ow_t": nc.dram_tensor("ow_t", [FPC, E], F16, kind="ExternalInput").ap(),
        "bias3": nc.dram_tensor("bias3", [FPC, 3], F32, kind="ExternalInput").ap(),
        "cpack": nc.dram_tensor("cpack", [128, 144], F16, kind="ExternalInput").ap(),
        "out_p": nc.dram_tensor("out_p", [T, E], F16, kind="ExternalOutput").ap(),
    }
    if DEBUG:
        io["dbg_qt"] = nc.dram_tensor("dbg_qt", [128, T], F16, kind="ExternalOutput").ap()
        io["dbg_kt"] = nc.dram_tensor("dbg_kt", [128, T], F16, kind="ExternalOutput").ap()
        io["dbg_at"] = nc.dram_tensor("dbg_at", [128, T], F16, kind="ExternalOutput").ap()
        io["dbg_v00"] = nc.dram_tensor("dbg_v00", [128, 16 * 65], F16, kind="ExternalOutput").ap()
    with tile.TileContext(nc) as tc:
        with ExitStack() as ctx:
            _build(ctx, tc, io)
    nc.compile()
    _CACHE["nc"] = nc
    return nc


def kernel(hidden_states, q_w, q_b, k_w, k_b, v_w, v_b, o_w, o_b):
    global LAST_RESULT
    nc = _get_program()

    f32c = lambda a: np.ascontiguousarray(a, dtype=np.float32)
    f16c = lambda a: np.ascontiguousarray(a, dtype=np.float16)
    # hs_p[p, tb, et, n] = hs[token tb*512+n, feature et*128+p]
    hs_t = np.asarray(hidden_states, dtype=np.float32).reshape(T, E).T
    hs_pm = f16c(hs_t.reshape(8, 128, 8, 512).transpose(1, 2, 0, 3))
    wp = lambda w, sl: f16c(
        np.asarray(w)[sl, :].T.reshape(8, 128, FPC).transpose(1, 0, 2)
    )
    in_maps = []
    for c in range(NCORE):
        sl = slice(c * FPC, (c + 1) * FPC)
        in_maps.append({
            "hs_p": hs_pm,
            "wq_p": wp(q_w, sl),
            "wk_p": wp(k_w, sl),
            "wv_p": wp(v_w, sl),
            "ow_t": f16c(np.asarray(o_w)[:, sl].T),
            "bias3": f32c(np.stack([np.asarray(q_b)[sl], np.asarray(k_b)[sl],
                                     np.asarray(v_b)[sl]], axis=1)),
            "cpack": f16c(np.concatenate([np.eye(128, dtype=np.float16),
                                          np.ones((128, 16), np.float16)], axis=1)),
        })

    res = run_bass_kernel_spmd(nc, in_maps, list(range(NCORE)), trace=TRACE)
    LAST_RESULT = res
    out = res.results[0]["out_p"].astype(np.float64)
    for c in range(1, NCORE):
        out += res.results[c]["out_p"]
    out += np.asarray(o_b, dtype=np.float64)
    return out.reshape(B, S, E).astype(np.float32)


# revision 14
# speedup vs baseline: 1.4565x; 1.1210x over previous
"""Trainium2 Bass kernel for a dense multi-head self-attention block.

Computation (matches torch/diffusers Attention with upcast softmax):
    q/k/v = hs @ W.T + b ; per-head scaled QK^T ; softmax ; PV ; out proj.
Shapes: hs [2, 2048, 1024], 16 heads x 64 dim, fp32 in/out.

Sharding: batch*head parallel over 8 cores. Core c owns heads {2c, 2c+1}
(feature slice c*128:(c+1)*128 of E) for both batches. The host pre-packs
hidden_states and weights into partition-major fp16 layouts (one contiguous
run per SBUF partition per DMA), so the device never transposes activations
and DMA descriptor counts stay low. Per core:
  - Q^T/K^T/V^T projections for its 128 features over all 4096 tokens
    (fp16 operands, fp32 PSUM accumulation),
  - V^T is re-tiled to [tokens, features] via PE transposes; an all-ones
    column is appended so the PV matmul also accumulates the softmax
    denominator (row 64 of each PV accumulator),
  - attention in scores^T layout (K @ Q^T: k-tokens on partitions, q on
    the free dim); QK row-alternates the two heads so LDWEIGHTS hides
    under the previous matmul (PE is N-cycle-bound: ~0.50 ns per moving
    row at 512-wide PSUM tiles),
  - softmax exp is split across two engines to keep it off the PE's
    critical path: ~2/3 of (kt, head) tiles run exact exp on ScalarE
    (scale folded in); ~1/3 run on DVE as a Schraudolph-style bit-trick:
    i16 = round(s * 1024*log2(e)*SCALE + (15360 + c)), bitcast to fp16
    gives 2^x with <3% mantissa-interp error (end-to-end rel err ~6e-3,
    tolerance 2e-2). No max-subtraction: scores are O(1) by construction.
  - softmax normalization: the denominator row (PV row 64) is
    reciprocal'd in place on the same partition (free-dim offset), then
    DMA-broadcast to 64 partitions; fused into the PSUM->SBUF copy path,
  - partial out-projection (contraction over this core's 128 features)
    written as fp16 [4096, 1024]; the host sums the 8 partials + o_b.
"""

import numpy as np

import concourse.bass as bass
import concourse.mybir as mybir
import concourse.tile as tile
from concourse import bacc
from concourse.bass_utils import run_bass_kernel_spmd

B, S, E = 2, 2048, 1024
H, D = 16, 64
SCALE = D ** -0.5
NCORE = 8
T = B * S              # 4096 tokens
FPC = 128              # features per core (2 heads x 64)
HPC = 2                # heads per core

F32 = mybir.dt.float32
F16 = mybir.dt.float16
I16 = mybir.dt.int16
EXP = mybir.ActivationFunctionType.Exp
MULT = mybir.AluOpType.mult
ADD = mybir.AluOpType.add

# Schraudolph exp2 constants (fp16 bit trick), scale folded in:
#   i16 = s * SCALE * 1024/ln(2) + 15360 + c
SCH_A = SCALE * 1024.0 / float(np.log(2.0))
SCH_C = 15360.0 - 45.0

# set by test harness to profile; results stashed in LAST_RESULT
TRACE = False
DEBUG = False
LAST_RESULT = None
_CACHE = {}


def _exp_engine(kt, h):
    """Which engine computes exp for (kt, head) tile: ~1/3 DVE, rest ACT."""
    return "dve" if (kt * 2 + h) % 3 == 1 else "act"


def _build(ctx, tc, io):
    nc = tc.nc
    hs_p, wq_p, wk_p, wv_p, ow_t, out_p = (
        io["hs_p"], io["wq_p"], io["wk_p"], io["wv_p"], io["ow_t"], io["out_p"],
    )

    # ---------------- pools ----------------
    consts = ctx.enter_context(tc.tile_pool(name="consts", bufs=1))
    persist = ctx.enter_context(tc.tile_pool(name="persist", bufs=1))
    hst_pool = ctx.enter_context(tc.tile_pool(name="hst", bufs=4))
    vt_pool = ctx.enter_context(tc.tile_pool(name="vt", bufs=3))
    pt_pool = ctx.enter_context(tc.tile_pool(name="pt", bufs=6))
    bc_pool = ctx.enter_context(tc.tile_pool(name="bcs", bufs=3))
    rc_pool = ctx.enter_context(tc.tile_pool(name="rc", bufs=2))
    out_pool = ctx.enter_context(tc.tile_pool(name="outs", bufs=8))
    dr_pool = ctx.enter_context(tc.tile_pool(name="drb", bufs=2, space="DRAM"))
    # PSUM: 8 banks total. p_big = 2x[128,1024] (4 banks),
    # p_acc = 4x[128,512] (4 banks)
    p_big = ctx.enter_context(tc.tile_pool(name="p_big", bufs=2, space="PSUM"))
    p_acc = ctx.enter_context(tc.tile_pool(name="p_acc", bufs=4, space="PSUM"))

    # ---------------- constants / weights ----------------
    wq_sb = consts.tile([128, 8, 128], F16, tag="wq")
    wk_sb = consts.tile([128, 8, 128], F16, tag="wk")
    wv_sb = consts.tile([128, 8, 128], F16, tag="wv")
    ow_sb = consts.tile([128, 1024], F16, tag="ow")
    bias_sb = consts.tile([128, 3], F32, tag="bias")
    qb_sb, kb_sb, vb_sb = bias_sb[:, 0:1], bias_sb[:, 1:2], bias_sb[:, 2:3]
    cpack = consts.tile([128, 144], F16, tag="cpack")
    ident = cpack[:, 0:128]

    # wq + bias first (the first matmuls only need these + hst0's low half;
    # remaining consts are queued behind the first hidden-state chunks)
    nc.sync.dma_start(wq_sb[:], wq_p[:])
    nc.sync.dma_start(bias_sb[:], io["bias3"][:])

    # persistent activations: feature dim (128 = 2 heads x 64) on partitions
    qt_sb = persist.tile([128, T], F16, tag="qt")      # Q^T
    kt_sb = persist.tile([128, T], F16, tag="kt")      # K^T
    # attn out^T (normalized), one tile per 1024-q block so the out-proj
    # only waits on the q-blocks it actually reads
    at4 = [
        persist.tile([128, 1024], F16, tag=f"at{i}", name=f"at{i}")
        for i in range(4)
    ]
    v_bh = [
        [
            persist.tile([128, 16, 65], F16, tag=f"v{b}{h}", name=f"v{b}{h}")
            for h in range(2)
        ]
        for b in range(B)
    ]
    # ---------------- phase 1: QKV projections ----------------
    for tb in range(8):                      # 512-token blocks over B*S
        # two half-tiles per token block so the first matmuls (et 0..3)
        # start as soon as the low half lands
        hst_a = hst_pool.tile([128, 4, 512], F16, tag="hsta", name="hst_a")
        nc.sync.dma_start(hst_a[:], hs_p[:, tb, 0:4])
        hst_b = hst_pool.tile([128, 4, 512], F16, tag="hstb", name="hst_b")
        nc.sync.dma_start(hst_b[:], hs_p[:, tb, 4:8])
        if tb == 0:
            nc.sync.dma_start(wk_sb[:], wk_p[:])
            nc.sync.dma_start(wv_sb[:], wv_p[:])
            nc.sync.dma_start(cpack[:], io["cpack"][:])
            nc.sync.dma_start(ow_sb[:], ow_t[:])
        hs_et = lambda et: (hst_a if et < 4 else hst_b)[:, et % 4, :]
        for w_sb, b_sb, dest in ((wq_sb, qb_sb, qt_sb), (wk_sb, kb_sb, kt_sb)):
            ps = p_big.tile([128, 512], F32, tag="sc", name="ps")
            for et in range(8):
                nc.tensor.matmul(
                    ps[:], w_sb[:, et, :], hs_et(et),
                    start=(et == 0), stop=(et == 7),
                )
            nc.vector.tensor_scalar_add(
                dest[:, tb * 512:(tb + 1) * 512], ps[:], b_sb[:]
            )
        # V^T then transpose into [tokens, features] tiles
        vps = p_acc.tile([128, 512], F32, tag="acc")
        for et in range(8):
            nc.tensor.matmul(
                vps[:], wv_sb[:, et, :], hs_et(et),
                start=(et == 0), stop=(et == 7),
            )
        vtt = vt_pool.tile([128, 512], F16, tag="vtt")
        nc.vector.tensor_scalar_add(vtt[:], vps[:], vb_sb[:])
        b = tb // 4
        for j in range(4):
            ktl = (tb % 4) * 4 + j           # k-tile index within batch
            tps = p_acc.tile([128, 128], F16, tag="acc")
            nc.tensor.transpose(tps[:], vtt[:, j * 128:(j + 1) * 128], ident[:])
            nc.vector.tensor_copy(v_bh[b][0][:, ktl, 0:64], tps[:, 0:64])
            nc.vector.tensor_copy(v_bh[b][1][:, ktl, 0:64], tps[:, 64:128])

    # v_bh[b][h][:, kt, 0:64]: token kt*128+p of batch b, head-h features;
    # column 64 is all-ones (rides along in PV to accumulate softmax denom).
    # Emitted after phase 1 so the DVE queue isn't blocked on the cpack DMA
    # at startup; PV (the consumer) only starts in phase 2.
    for b in range(B):
        for h in range(2):
            nc.vector.tensor_copy(
                v_bh[b][h][:, :, 64:65],
                cpack[:, 128:144].rearrange("p (a o) -> p a o", o=1),
            )

    # ---------------- phase 2: attention ----------------
    # The normalization chain of q-block N (reciprocal pack-dance, DMA
    # broadcast, muls) is deferred into q-block N+1's kt loop so its DMA
    # latency never blocks the DVE queue entries that feed the PE.
    def emit_norm(pvs_all, at_dst):
        pvs = [pvs_all[:, 0:1024], pvs_all[:, 1024:2048]]
        # Reciprocal of the 2048 denominators (2 heads x 1024 q).
        # DVE reciprocal costs ~6.3 ns per free-dim element regardless of
        # partition count, so pack them across 128 partitions via a DRAM
        # bounce: [1,2048] row -> [128,16] -> recip -> row -> broadcast.
        den_dr = dr_pool.tile([2, 1024], F32, tag="den_dr", name="den_dr")
        nc.sync.dma_start(den_dr.rearrange("a n -> (a n)"), pvs_all[64:65, :])
        dpack = rc_pool.tile([128, 16], F32, tag="rc", name="dpack")
        nc.sync.dma_start(
            dpack[:],
            den_dr.rearrange("a n -> (a n)").rearrange("(p i) -> p i", p=128),
        )
        rpack = rc_pool.tile([128, 16], F32, tag="rc", name="rpack")
        with nc.allow_low_precision(reason="softmax denom reciprocal"):
            nc.vector.reciprocal(rpack[:], dpack[:])
        rcp_dr = dr_pool.tile([2, 1024], F32, tag="rcp_dr", name="rcp_dr")
        nc.sync.dma_start(
            rcp_dr.rearrange("a n -> (a n)").rearrange("(p i) -> p i", p=128),
            rpack[:],
        )
        bc = [None, None]
        for h in range(2):
            bch = bc_pool.tile([64, 1024], F32, tag="bcs", name=f"bc{h}")
            nc.sync.dma_start(bch[:], rcp_dr[h:h + 1, :].broadcast_to([64, 1024]))
            bc[h] = bch
        nc.vector.tensor_mul(at_dst[0:64, :], pvs[0][0:64, :], bc[0][:])
        a1 = vt_pool.tile([64, 1024], F16, tag="a1", name="a1")
        nc.vector.tensor_mul(a1[:], pvs[1][0:64, :], bc[1][:])
        # head 1 lives on partitions 64:128 of at -- shift via SBUF->SBUF DMA
        nc.sync.dma_start(at_dst[64:128, :], a1[:])

    pending_norm = None
    for b in range(B):
        toff = b * S
        for qb_i in range(2):                # 1024-wide q blocks
            qbg = b * 2 + qb_i               # global q-block index
            qoff = toff + qb_i * 1024
            pv = [
                [
                    p_acc.tile([65, 512], F32, tag="acc", name=f"pv{h}{qs}")
                    for qs in range(2)
                ]
                for h in range(2)
            ]
            def emit_qk(kt):
                koff2 = toff + kt * 128
                sc = [
                    p_big.tile([128, 1024], F32, tag="sc", name=f"sc{h}")
                    for h in range(2)
                ]
                # alternate head row-groups so each LDWEIGHTS overlaps the
                # previous matmul (different row group -> PE pulls it ahead)
                for qs in range(2):
                    for h in range(2):
                        p0 = h * 64
                        nc.tensor.matmul(
                            sc[h][:, qs * 512:(qs + 1) * 512],
                            kt_sb[p0:p0 + 64, koff2:koff2 + 128],
                            qt_sb[p0:p0 + 64, qoff + qs * 512:qoff + (qs + 1) * 512],
                            start=True, stop=True,
                        )
                return sc

            # software pipeline: iteration kt computes exp(kt), QK(kt+1) and
            # PV(kt-1) -- the PV consuming pt(kt-1) has had a full iteration
            # (~1.7us) of slack, so the PE never waits on the ~1.3us exp.
            def emit_pv(kt, pt):
                first, last = kt == 0, kt == 15
                for qs in range(2):
                    q0, q1 = qs * 512, (qs + 1) * 512
                    # PV with ones-row: out rows 0:64 = V^T P^T, row 64 = denom
                    for h in range(2):
                        nc.tensor.matmul(
                            pv[h][qs][:], v_bh[b][h][:, kt, :],
                            pt[h][:, q0:q1], start=first, stop=last,
                        )

            sc_next = emit_qk(0)
            pt_prev = None
            for kt in range(16):
                sc = sc_next
                pt = []
                for h in range(2):
                    pth = pt_pool.tile([128, 1024], F16, tag="pt")
                    if _exp_engine(kt, h) == "act":
                        nc.scalar.activation(pth[:], sc[h][:], EXP, scale=SCALE)
                    else:
                        nc.vector.tensor_scalar(
                            pth[:].bitcast(I16), sc[h][:], SCH_A, SCH_C,
                            MULT, ADD,
                        )
                    pt.append(pth)
                if kt == 3 and pending_norm is not None:
                    emit_norm(*pending_norm)
                    pending_norm = None
                if kt < 15:
                    sc_next = emit_qk(kt + 1)
                if pt_prev is not None:
                    emit_pv(kt - 1, pt_prev)
                pt_prev = pt
            emit_pv(15, pt_prev)
            # copy PV out of PSUM immediately (frees the accumulator banks
            # for the next q-block); the normalization itself is deferred.
            pvs_all = rc_pool.tile([65, 2048], F32, tag="pvs", name="pvs_all")
            pvs = [pvs_all[:, 0:1024], pvs_all[:, 1024:2048]]
            for h in range(2):
                for qs in range(2):
                    nc.vector.tensor_copy(
                        pvs[h][:, qs * 512:(qs + 1) * 512], pv[h][qs][:]
                    )
            pending_norm = (pvs_all, at4[qbg])
    emit_norm(*pending_norm)
    pending_norm = None

    if DEBUG:
        nc.sync.dma_start(io["dbg_qt"][:], qt_sb[:])
        nc.sync.dma_start(io["dbg_kt"][:], kt_sb[:])
        for i in range(4):
            nc.sync.dma_start(io["dbg_at"][:, i * 1024:(i + 1) * 1024], at4[i][:])
        nc.sync.dma_start(io["dbg_v00"][:], v_bh[0][0][:].rearrange("p a b -> p (a b)"))

    # ---------------- phase 3: partial out-projection ----------------
    # ops tiles alternate between both PSUM pools so up to 6 matmul/copy
    # pairs are in flight; copies alternate DVE / ScalarE.
    for tb in range(32):                     # 128-token blocks
        t0 = tb * 128
        at_t = at4[tb // 8]
        c0 = (tb % 8) * 128
        ot = out_pool.tile([128, 1024], F16, tag="outs", name="ot")
        for eb in range(2):
            pool = p_acc if (tb * 2 + eb) % 3 != 2 else p_big
            ops = pool.tile([128, 512], F32, tag="acc" if pool is p_acc else "sc",
                            name="ops")
            nc.tensor.matmul(
                ops[:], at_t[:, c0:c0 + 128],
                ow_sb[:, eb * 512:(eb + 1) * 512],
                start=True, stop=True,
            )
            if eb == 0:
                nc.vector.tensor_copy(ot[:, 0:512], ops[:])
            else:
                nc.scalar.copy(ot[:, 512:1024], ops[:])
        nc.sync.dma_start(out_p[t0:t0 + 128, :], ot[:])


def _get_program():
    if "nc" in _CACHE:
        return _CACHE["nc"]
    from contextlib import ExitStack

    nc = bacc.Bacc("TRN2", target_bir_lowering=False, debug=False,
                   num_devices=NCORE)
    io = {
        "hs_p": nc.dram_tensor("hs_p", [128, 8, 8, 512], F16, kind="ExternalInput").ap(),
        "wq_p": nc.dram_tensor("wq_p", [128, 8, 128], F16, kind="ExternalInput").ap(),
        "wk_p": nc.dram_tensor("wk_p", [128, 8, 128], F16, kind="ExternalInput").ap(),
        "wv_p": nc.dram_tensor("wv_p", [128, 8, 128], F16, kind="ExternalInput").ap(),
        "ow_t": nc.dram_tensor("ow_t", [FPC, E], F16, kind="ExternalInput").ap(),
        "bias3": nc.dram_tensor("bias3", [FPC, 3], F32, kind="ExternalInput").ap(),
        "cpack": nc.dram_tensor("cpack", [128, 144], F16, kind="ExternalInput").ap(),
        "out_p": nc.dram_tensor("out_p", [T, E], F16, kind="ExternalOutput").ap(),
    }
    if DEBUG:
        io["dbg_qt"] = nc.dram_tensor("dbg_qt", [128, T], F16, kind="ExternalOutput").ap()
        io["dbg_kt"] = nc.dram_tensor("dbg_kt", [128, T], F16, kind="ExternalOutput").ap()
        io["dbg_at"] = nc.dram_tensor("dbg_at", [128, T], F16, kind="ExternalOutput").ap()
        io["dbg_v00"] = nc.dram_tensor("dbg_v00", [128, 16 * 65], F16, kind="ExternalOutput").ap()
    with tile.TileContext(nc) as tc:
        with ExitStack() as ctx:
            _build(ctx, tc, io)
    nc.compile()
    _CACHE["nc"] = nc
    return nc


def kernel(hidden_states, q_w, q_b, k_w, k_b, v_w, v_b, o_w, o_b):
    global LAST_RESULT
    nc = _get_program()

    f32c = lambda a: np.ascontiguousarray(a, dtype=np.float32)
    f16c = lambda a: np.ascontiguousarray(a, dtype=np.float16)
    # hs_p[p, tb, et, n] = hs[token tb*512+n, feature et*128+p]
    hs_t = np.asarray(hidden_states, dtype=np.float32).reshape(T, E).T
    hs_pm = f16c(hs_t.reshape(8, 128, 8, 512).transpose(1, 2, 0, 3))
    wp = lambda w, sl: f16c(
        np.asarray(w)[sl, :].T.reshape(8, 128, FPC).transpose(1, 0, 2)
    )
    in_maps = []
    for c in range(NCORE):
        sl = slice(c * FPC, (c + 1) * FPC)
        in_maps.append({
            "hs_p": hs_pm,
            "wq_p": wp(q_w, sl),
            "wk_p": wp(k_w, sl),
            "wv_p": wp(v_w, sl),
            "ow_t": f16c(np.asarray(o_w)[:, sl].T),
            "bias3": f32c(np.stack([np.asarray(q_b)[sl], np.asarray(k_b)[sl],
                                     np.asarray(v_b)[sl]], axis=1)),
            "cpack": f16c(np.concatenate([np.eye(128, dtype=np.float16),
                                          np.ones((128, 16), np.float16)], axis=1)),
        })

    res = run_bass_kernel_spmd(nc, in_maps, list(range(NCORE)), trace=TRACE)
    LAST_RESULT = res
    out = res.results[0]["out_p"].astype(np.float64)
    for c in range(1, NCORE):
        out += res.results[c]["out_p"]
    out += np.asarray(o_b, dtype=np.float64)
    return out.reshape(B, S, E).astype(np.float32)


# revision 16
# speedup vs baseline: 1.4889x; 1.0223x over previous
"""Trainium2 Bass kernel for a dense multi-head self-attention block.

Computation (matches torch/diffusers Attention with upcast softmax):
    q/k/v = hs @ W.T + b ; per-head scaled QK^T ; softmax ; PV ; out proj.
Shapes: hs [2, 2048, 1024], 16 heads x 64 dim, fp32 in/out.

Sharding: batch*head parallel over 8 cores. Core c owns heads {2c, 2c+1}
(feature slice c*128:(c+1)*128 of E) for both batches. The host pre-packs
hidden_states and weights into partition-major fp16 layouts (one contiguous
run per SBUF partition per DMA), so the device never transposes activations
and DMA descriptor counts stay low. Per core:
  - Q^T/K^T/V^T projections for its 128 features over all 4096 tokens
    (fp16 operands, fp32 PSUM accumulation),
  - V^T is re-tiled to [tokens, features] via PE transposes; an all-ones
    column is appended so the PV matmul also accumulates the softmax
    denominator (row 64 of each PV accumulator),
  - attention in scores^T layout (K @ Q^T: k-tokens on partitions, q on
    the free dim); QK row-alternates the two heads so LDWEIGHTS hides
    under the previous matmul (PE is N-cycle-bound: ~0.50 ns per moving
    row at 512-wide PSUM tiles),
  - softmax exp is split across two engines to keep it off the PE's
    critical path: ~2/3 of (kt, head) tiles run exact exp on ScalarE
    (scale folded in); ~1/3 run on DVE as a Schraudolph-style bit-trick:
    i16 = round(s * 1024*log2(e)*SCALE + (15360 + c)), bitcast to fp16
    gives 2^x with <3% mantissa-interp error (end-to-end rel err ~6e-3,
    tolerance 2e-2). No max-subtraction: scores are O(1) by construction.
  - softmax normalization: the denominator row (PV row 64) is
    reciprocal'd in place on the same partition (free-dim offset), then
    DMA-broadcast to 64 partitions; fused into the PSUM->SBUF copy path,
  - partial out-projection (contraction over this core's 128 features)
    written as fp16 [4096, 1024]; the host sums the 8 partials + o_b.
"""

import numpy as np

import concourse.bass as bass
import concourse.mybir as mybir
import concourse.tile as tile
from concourse import bacc
from concourse.bass_utils import run_bass_kernel_spmd

B, S, E = 2, 2048, 1024
H, D = 16, 64
SCALE = D ** -0.5
NCORE = 8
T = B * S              # 4096 tokens
FPC = 128              # features per core (2 heads x 64)
HPC = 2                # heads per core

F32 = mybir.dt.float32
F16 = mybir.dt.float16
I16 = mybir.dt.int16
EXP = mybir.ActivationFunctionType.Exp
MULT = mybir.AluOpType.mult
ADD = mybir.AluOpType.add

# Schraudolph exp2 constants (fp16 bit trick), scale folded in:
#   i16 = s * SCALE * 1024/ln(2) + 15360 + c
SCH_A = SCALE * 1024.0 / float(np.log(2.0))
SCH_C = 15360.0 - 45.0

# set by test harness to profile; results stashed in LAST_RESULT
TRACE = False
DEBUG = False
LAST_RESULT = None
_CACHE = {}


def _exp_engine(kt, h):
    """Which engine computes exp for (kt, head) tile: ~1/3 DVE, rest ACT."""
    return "dve" if (kt * 2 + h) % 3 == 1 else "act"


def _build(ctx, tc, io):
    nc = tc.nc
    hs_p, wq_p, wk_p, wv_p, ow_t, out_p = (
        io["hs_p"], io["wq_p"], io["wk_p"], io["wv_p"], io["ow_t"], io["out_p"],
    )

    # ---------------- pools ----------------
    consts = ctx.enter_context(tc.tile_pool(name="consts", bufs=1))
    persist = ctx.enter_context(tc.tile_pool(name="persist", bufs=1))
    hst_pool = ctx.enter_context(tc.tile_pool(name="hst", bufs=4))
    vt_pool = ctx.enter_context(tc.tile_pool(name="vt", bufs=3))
    pt_pool = ctx.enter_context(tc.tile_pool(name="pt", bufs=6))
    bc_pool = ctx.enter_context(tc.tile_pool(name="bcs", bufs=3))
    rc_pool = ctx.enter_context(tc.tile_pool(name="rc", bufs=2))
    out_pool = ctx.enter_context(tc.tile_pool(name="outs", bufs=8))
    dr_pool = ctx.enter_context(tc.tile_pool(name="drb", bufs=2, space="DRAM"))
    # PSUM: 8 banks total. p_big = 2x[128,1024] (4 banks),
    # p_acc = 4x[128,512] (4 banks)
    p_big = ctx.enter_context(tc.tile_pool(name="p_big", bufs=2, space="PSUM"))
    p_acc = ctx.enter_context(tc.tile_pool(name="p_acc", bufs=4, space="PSUM"))

    # ---------------- constants / weights ----------------
    wq_sb = consts.tile([128, 8, 128], F16, tag="wq")
    wk_sb = consts.tile([128, 8, 128], F16, tag="wk")
    wv_sb = consts.tile([128, 8, 128], F16, tag="wv")
    ow_sb = consts.tile([128, 1024], F16, tag="ow")
    bias_sb = consts.tile([128, 3], F32, tag="bias")
    qb_sb, kb_sb, vb_sb = bias_sb[:, 0:1], bias_sb[:, 1:2], bias_sb[:, 2:3]
    cpack = consts.tile([128, 144], F16, tag="cpack")
    ident = cpack[:, 0:128]

    # wq + bias first (the first matmuls only need these + hst0's low half;
    # remaining consts are queued behind the first hidden-state chunks)
    nc.sync.dma_start(wq_sb[:], wq_p[:])
    nc.sync.dma_start(bias_sb[:], io["bias3"][:])

    # persistent activations: feature dim (128 = 2 heads x 64) on partitions
    qt_sb = persist.tile([128, T], F16, tag="qt")      # Q^T
    kt_sb = persist.tile([128, T], F16, tag="kt")      # K^T
    # attn out^T (normalized), one tile per 1024-q block so the out-proj
    # only waits on the q-blocks it actually reads
    at4 = [
        persist.tile([128, 1024], F16, tag=f"at{i}", name=f"at{i}")
        for i in range(4)
    ]
    v_bh = [
        [
            persist.tile([128, 16, 65], F16, tag=f"v{b}{h}", name=f"v{b}{h}")
            for h in range(2)
        ]
        for b in range(B)
    ]
    # ---------------- phase 1: QKV projections ----------------
    for tb in range(8):                      # 512-token blocks over B*S
        # two half-tiles per token block so the first matmuls (et 0..3)
        # start as soon as the low half lands
        hst_a = hst_pool.tile([128, 4, 512], F16, tag="hsta", name="hst_a")
        nc.sync.dma_start(hst_a[:], hs_p[:, tb, 0:4])
        hst_b = hst_pool.tile([128, 4, 512], F16, tag="hstb", name="hst_b")
        nc.sync.dma_start(hst_b[:], hs_p[:, tb, 4:8])
        if tb == 0:
            nc.sync.dma_start(wk_sb[:], wk_p[:])
            nc.sync.dma_start(wv_sb[:], wv_p[:])
            nc.sync.dma_start(cpack[:], io["cpack"][:])
            nc.sync.dma_start(ow_sb[:], ow_t[:])
        hs_et = lambda et: (hst_a if et < 4 else hst_b)[:, et % 4, :]
        for w_sb, b_sb, dest in ((wq_sb, qb_sb, qt_sb), (wk_sb, kb_sb, kt_sb)):
            ps = p_big.tile([128, 512], F32, tag="sc", name="ps")
            for et in range(8):
                nc.tensor.matmul(
                    ps[:], w_sb[:, et, :], hs_et(et),
                    start=(et == 0), stop=(et == 7),
                )
            nc.vector.tensor_scalar_add(
                dest[:, tb * 512:(tb + 1) * 512], ps[:], b_sb[:]
            )
        # V^T then transpose into [tokens, features] tiles
        vps = p_acc.tile([128, 512], F32, tag="acc")
        for et in range(8):
            nc.tensor.matmul(
                vps[:], wv_sb[:, et, :], hs_et(et),
                start=(et == 0), stop=(et == 7),
            )
        vtt = vt_pool.tile([128, 512], F16, tag="vtt")
        nc.vector.tensor_scalar_add(vtt[:], vps[:], vb_sb[:])
        b = tb // 4
        for j in range(4):
            ktl = (tb % 4) * 4 + j           # k-tile index within batch
            tps = p_acc.tile([128, 128], F16, tag="acc")
            nc.tensor.transpose(tps[:], vtt[:, j * 128:(j + 1) * 128], ident[:])
            nc.vector.tensor_copy(v_bh[b][0][:, ktl, 0:64], tps[:, 0:64])
            nc.vector.tensor_copy(v_bh[b][1][:, ktl, 0:64], tps[:, 64:128])

    # v_bh[b][h][:, kt, 0:64]: token kt*128+p of batch b, head-h features;
    # column 64 is all-ones (rides along in PV to accumulate softmax denom).
    # Emitted after phase 1 so the DVE queue isn't blocked on the cpack DMA
    # at startup; PV (the consumer) only starts in phase 2.
    for b in range(B):
        for h in range(2):
            nc.vector.tensor_copy(
                v_bh[b][h][:, :, 64:65],
                cpack[:, 128:144].rearrange("p (a o) -> p a o", o=1),
            )

    # ---------------- phase 2: attention ----------------
    # The normalization chain of q-block N (reciprocal pack-dance, DMA
    # broadcast, muls) is deferred into q-block N+1's kt loop so its DMA
    # latency never blocks the DVE queue entries that feed the PE.
    def emit_norm(pvs_all, at_dst):
        pvs = [pvs_all[:, 0:1024], pvs_all[:, 1024:2048]]
        # Reciprocal of the 2048 denominators (2 heads x 1024 q).
        # DVE reciprocal costs ~6.3 ns per free-dim element regardless of
        # partition count, so pack them across 128 partitions via a DRAM
        # bounce: [1,2048] row -> [128,16] -> recip -> row -> broadcast.
        den_dr = dr_pool.tile([2, 1024], F32, tag="den_dr", name="den_dr")
        nc.sync.dma_start(den_dr.rearrange("a n -> (a n)"), pvs_all[64:65, :])
        dpack = rc_pool.tile([128, 16], F32, tag="rc", name="dpack")
        nc.sync.dma_start(
            dpack[:],
            den_dr.rearrange("a n -> (a n)").rearrange("(p i) -> p i", p=128),
        )
        rpack = rc_pool.tile([128, 16], F32, tag="rc", name="rpack")
        with nc.allow_low_precision(reason="softmax denom reciprocal"):
            nc.vector.reciprocal(rpack[:], dpack[:])
        rcp_dr = dr_pool.tile([2, 1024], F32, tag="rcp_dr", name="rcp_dr")
        nc.sync.dma_start(
            rcp_dr.rearrange("a n -> (a n)").rearrange("(p i) -> p i", p=128),
            rpack[:],
        )
        bc = [None, None]
        for h in range(2):
            bch = bc_pool.tile([64, 1024], F32, tag="bcs", name=f"bc{h}")
            nc.sync.dma_start(bch[:], rcp_dr[h:h + 1, :].broadcast_to([64, 1024]))
            bc[h] = bch
        nc.vector.tensor_mul(at_dst[0:64, :], pvs[0][0:64, :], bc[0][:])
        a1 = vt_pool.tile([64, 1024], F16, tag="a1", name="a1")
        nc.vector.tensor_mul(a1[:], pvs[1][0:64, :], bc[1][:])
        # head 1 lives on partitions 64:128 of at -- shift via SBUF->SBUF DMA
        nc.sync.dma_start(at_dst[64:128, :], a1[:])

    # One flat software pipeline over all 64 (q-block, kt) slots: slot s
    # emits exp(s), QK(s+1) and PV(s-1). Crossing q-block boundaries keeps
    # the PE streaming while the boundary exp/copies resolve.
    pending_norm = None
    pv_cur = pv_prev = None
    SLOTS = [(b, qb_i, kt) for b in range(B) for qb_i in range(2)
             for kt in range(16)]

    def emit_qk(slot):
        b, qb_i, kt = slot
        toff = b * S
        qoff = toff + qb_i * 1024
        koff2 = toff + kt * 128
        sc = [
            p_big.tile([128, 1024], F32, tag="sc", name=f"sc{h}")
            for h in range(2)
        ]
        # alternate head row-groups so each LDWEIGHTS overlaps the
        # previous matmul (different row group -> PE pulls it ahead)
        for qs in range(2):
            for h in range(2):
                p0 = h * 64
                nc.tensor.matmul(
                    sc[h][:, qs * 512:(qs + 1) * 512],
                    kt_sb[p0:p0 + 64, koff2:koff2 + 128],
                    qt_sb[p0:p0 + 64, qoff + qs * 512:qoff + (qs + 1) * 512],
                    start=True, stop=True,
                )
        return sc

    def emit_pv(slot, pt, pv):
        b, qb_i, kt = slot
        first, last = kt == 0, kt == 15
        for qs in range(2):
            q0, q1 = qs * 512, (qs + 1) * 512
            # PV with ones-row: out rows 0:64 = V^T P^T, row 64 = denom
            for h in range(2):
                nc.tensor.matmul(
                    pv[h][qs][:], v_bh[b][h][:, kt, :],
                    pt[h][:, q0:q1], start=first, stop=last,
                )

    def emit_pv_copies(qbg, pv):
        # copy PV out of PSUM (frees accumulator banks for the next
        # q-block), split DVE/ScalarE; normalization itself is deferred.
        pvs_all = rc_pool.tile([65, 2048], F32, tag="pvs", name="pvs_all")
        pvs = [pvs_all[:, 0:1024], pvs_all[:, 1024:2048]]
        for h in range(2):
            for qs in range(2):
                dst = pvs[h][:, qs * 512:(qs + 1) * 512]
                if qs == 0:
                    nc.vector.tensor_copy(dst, pv[h][qs][:])
                else:
                    nc.scalar.copy(dst, pv[h][qs][:])
        return (pvs_all, at4[qbg])

    sc_next = emit_qk(SLOTS[0])
    pt_prev = prev_slot = None
    for s, slot in enumerate(SLOTS):
        b, qb_i, kt = slot
        sc = sc_next
        if kt == 0:
            pv_cur = [
                [
                    p_acc.tile([65, 512], F32, tag="acc", name=f"pv{h}{qs}")
                    for qs in range(2)
                ]
                for h in range(2)
            ]
        pt = []
        for h in range(2):
            pth = pt_pool.tile([128, 1024], F16, tag="pt")
            if _exp_engine(kt, h) == "act":
                nc.scalar.activation(pth[:], sc[h][:], EXP, scale=SCALE)
            else:
                nc.vector.tensor_scalar(
                    pth[:].bitcast(I16), sc[h][:], SCH_A, SCH_C,
                    MULT, ADD,
                )
            pt.append(pth)
        if kt == 3 and pending_norm is not None:
            emit_norm(*pending_norm)
            pending_norm = None
        if s + 1 < len(SLOTS):
            sc_next = emit_qk(SLOTS[s + 1])
        if prev_slot is not None:
            emit_pv(prev_slot, pt_prev, pv_prev)
            if prev_slot[2] == 15:
                pending_norm = emit_pv_copies(
                    prev_slot[0] * 2 + prev_slot[1], pv_prev)
        pt_prev, prev_slot, pv_prev = pt, slot, pv_cur
    emit_pv(prev_slot, pt_prev, pv_prev)
    pending_norm_last = emit_pv_copies(3, pv_prev)
    assert pending_norm is None  # qbg 2's norm drained at slot (1,1,3)
    emit_norm(*pending_norm_last)

    if DEBUG:
        nc.sync.dma_start(io["dbg_qt"][:], qt_sb[:])
        nc.sync.dma_start(io["dbg_kt"][:], kt_sb[:])
        for i in range(4):
            nc.sync.dma_start(io["dbg_at"][:, i * 1024:(i + 1) * 1024], at4[i][:])
        nc.sync.dma_start(io["dbg_v00"][:], v_bh[0][0][:].rearrange("p a b -> p (a b)"))

    # ---------------- phase 3: partial out-projection ----------------
    # ops tiles alternate between both PSUM pools so up to 6 matmul/copy
    # pairs are in flight; copies alternate DVE / ScalarE.
    for tb in range(32):                     # 128-token blocks
        t0 = tb * 128
        at_t = at4[tb // 8]
        c0 = (tb % 8) * 128
        ot = out_pool.tile([128, 1024], F16, tag="outs", name="ot")
        for eb in range(2):
            pool = p_acc if (tb * 2 + eb) % 3 != 2 else p_big
            ops = pool.tile([128, 512], F32, tag="acc" if pool is p_acc else "sc",
                            name="ops")
            nc.tensor.matmul(
                ops[:], at_t[:, c0:c0 + 128],
                ow_sb[:, eb * 512:(eb + 1) * 512],
                start=True, stop=True,
            )
            if eb == 0:
                nc.vector.tensor_copy(ot[:, 0:512], ops[:])
            else:
                nc.scalar.copy(ot[:, 512:1024], ops[:])
        nc.sync.dma_start(out_p[t0:t0 + 128, :], ot[:])


def _get_program():
    if "nc" in _CACHE:
        return _CACHE["nc"]
    from contextlib import ExitStack

    nc = bacc.Bacc("TRN2", target_bir_lowering=False, debug=False,
                   num_devices=NCORE)
    io = {
        "hs_p": nc.dram_tensor("hs_p", [128, 8, 8, 512], F16, kind="ExternalInput").ap(),
        "wq_p": nc.dram_tensor("wq_p", [128, 8, 128], F16, kind="ExternalInput").ap(),
        "wk_p": nc.dram_tensor("wk_p", [128, 8, 128], F16, kind="ExternalInput").ap(),
        "wv_p": nc.dram_tensor("wv_p", [128, 8, 128], F16, kind="ExternalInput").ap(),
        "ow_t": nc.dram_tensor("ow_t", [FPC, E], F16, kind="ExternalInput").ap(),
        "bias3": nc.dram_tensor("bias3", [FPC, 3], F32, kind="ExternalInput").ap(),
        "cpack": nc.dram_tensor("cpack", [128, 144], F16, kind="ExternalInput").ap(),
        "out_p": nc.dram_tensor("out_p", [T, E], F16, kind="ExternalOutput").ap(),
    }
    if DEBUG:
        io["dbg_qt"] = nc.dram_tensor("dbg_qt", [128, T], F16, kind="ExternalOutput").ap()
        io["dbg_kt"] = nc.dram_tensor("dbg_kt", [128, T], F16, kind="ExternalOutput").ap()
        io["dbg_at"] = nc.dram_tensor("dbg_at", [128, T], F16, kind="ExternalOutput").ap()
        io["dbg_v00"] = nc.dram_tensor("dbg_v00", [128, 16 * 65], F16, kind="ExternalOutput").ap()
    with tile.TileContext(nc) as tc:
        with ExitStack() as ctx:
            _build(ctx, tc, io)
    nc.compile()
    _CACHE["nc"] = nc
    return nc


def kernel(hidden_states, q_w, q_b, k_w, k_b, v_w, v_b, o_w, o_b):
    global LAST_RESULT
    nc = _get_program()

    f32c = lambda a: np.ascontiguousarray(a, dtype=np.float32)
    f16c = lambda a: np.ascontiguousarray(a, dtype=np.float16)
    # hs_p[p, tb, et, n] = hs[token tb*512+n, feature et*128+p]
    hs_t = np.asarray(hidden_states, dtype=np.float32).reshape(T, E).T
    hs_pm = f16c(hs_t.reshape(8, 128, 8, 512).transpose(1, 2, 0, 3))
    wp = lambda w, sl: f16c(
        np.asarray(w)[sl, :].T.reshape(8, 128, FPC).transpose(1, 0, 2)
    )
    in_maps = []
    for c in range(NCORE):
        sl = slice(c * FPC, (c + 1) * FPC)
        in_maps.append({
            "hs_p": hs_pm,
            "wq_p": wp(q_w, sl),
            "wk_p": wp(k_w, sl),
            "wv_p": wp(v_w, sl),
            "ow_t": f16c(np.asarray(o_w)[:, sl].T),
            "bias3": f32c(np.stack([np.asarray(q_b)[sl], np.asarray(k_b)[sl],
                                     np.asarray(v_b)[sl]], axis=1)),
            "cpack": f16c(np.concatenate([np.eye(128, dtype=np.float16),
                                          np.ones((128, 16), np.float16)], axis=1)),
        })

    res = run_bass_kernel_spmd(nc, in_maps, list(range(NCORE)), trace=TRACE)
    LAST_RESULT = res
    out = res.results[0]["out_p"].astype(np.float64)
    for c in range(1, NCORE):
        out += res.results[c]["out_p"]
    out += np.asarray(o_b, dtype=np.float64)
    return out.reshape(B, S, E).astype(np.float32)
